# revision 50
# baseline (speedup 1.0000x reference)
"""Trainium2 Bass kernel for nn_Block_16544214024520 (dense_cnn).

Data-parallel over batch: 16 samples -> 2 per NeuronCore x 8 cores.
All parameters replicated. Per-sample layout: channels on partitions
(256 = 2 chunks of 128), pixels (64x64 = 4096) on the free dim.

Reference pipeline (per sample):
  gn(32) -> 1x1 conv(256->256)+silu -> gn(16) -> 3x3 grouped conv
  (g=4, 256->512)+silu -> gn(2) -> window-mean(8x8) -> radix amax ->
  1x1 g-conv(256->64)+silu -> gn(8) -> 1x1 g-conv(64->512) ->
  softmax over radix(2) -> gated combine -> channel matmul(256->256)
  -> gn(32) -> +residual

v2 design notes:
  - conv1 out-channels are permuted within each group to
    [r=0 c-block | r=1 c-block] so the radix amax / softmax are
    partition-offset ops (no PE transposes anywhere).
  - conv1 runs as 4 tap-pair matmuls (K=128) + 1 single (K=64 padded)
    per strip, using per-group input tiles that hold [x | x-shifted]
    on the partition dim (shifted copies built by DMA + vector).
  - group-norm stats: means ride the activation accumulators of the
    psum evacuations; sum-of-squares via scalar Square acts or vector
    tensor_tensor_reduce, balanced across engines.
  - gating is one 4x-mode scalar_tensor_tensor per (group, sample).
  - bf16 input only (residual add in bf16), bf16 DRAM output
    (host casts to fp32).
  - the two samples are interleaved so the attn latency chain of one
    overlaps the conv matmuls of the other.
"""

import os
import sys

for _p in ("/opt/trn_rl_repo", "/opt/pypackages"):
    if _p not in sys.path:
        sys.path.append(_p)

import ml_dtypes
import numpy as np

import concourse.bass as bass  # noqa: F401
import concourse.mybir as mybir
import concourse.tile as tile
from concourse import bacc

F32 = mybir.dt.float32
BF16 = mybir.dt.bfloat16
AF = mybir.ActivationFunctionType
ALU = mybir.AluOpType
AX = mybir.AxisListType

NCORES = 8
BPC = 2          # samples per core
C = 256          # channels
H = W = 64
NPIX = H * W     # 4096
PADW = W + 2     # 66
Hn = Wn = 8      # window grid
WS = 8           # window size
EPS = 1e-5
NT = 8           # n-tiles of 512 pixels
USE_SBUF_DMA = False
USE_ACT_ACCUM = True
USE_TTR = False


def _perm1():
    """conv1/conv3 out-channel permutation: within each 128-row group
    chunk g, rows [0:64] = (c=64g+p, r=0), rows [64:128] = (c, r=1).
    Original channel of (c, r) is 2c + r."""
    p = np.zeros(512, np.int64)
    for g in range(4):
        for q in range(64):
            p[128 * g + q] = 2 * (64 * g + q)          # r = 0
            p[128 * g + 64 + q] = 2 * (64 * g + q) + 1  # r = 1
    return p


# ---------------------------------------------------------------- host prep

def _host_consts():
    c = {}
    # GN1 (bn-mode): 32 groups of 8 over 256 ch
    gm1b = np.zeros((2, 128, 32), np.float32)
    rep1 = np.zeros((2, 128, 128), np.float32)
    for ch in range(2):
        for k in range(128):
            g = (128 * ch + k) // 8
            gm1b[ch, k, g] = 1.0 / 8.0
        for m in range(128):
            rep1[ch, (128 * ch + m) // 8 % 128, m] = 1.0
    c["gm1b"] = gm1b
    c["gm1s"] = gm1b / NPIX     # GN5 (sums-mode)
    c["rep1"] = rep1
    # GN2 (sums-mode): 16 groups of 16 over 256 ch
    gm2 = np.zeros((2, 128, 16), np.float32)
    rep2 = np.zeros((2, 128, 128), np.float32)
    for ch in range(2):
        for k in range(128):
            gm2[ch, k, (128 * ch + k) // 16] = 1.0 / (16.0 * NPIX)
        for m in range(128):
            rep2[ch, (128 * ch + m) // 16, m] = 1.0
    c["gm2"] = gm2
    c["rep2"] = rep2
    # GN3 (sums-mode): 2 groups of 256 over 512 ch; chunks 0,1 -> g0
    # (permutation within chunks doesn't change group membership)
    g3 = np.zeros((4, 128, 2), np.float32)
    r3 = np.zeros((4, 128, 128), np.float32)
    for mc in range(4):
        g3[mc, :, mc // 2] = 1.0 / (256.0 * NPIX)
        r3[mc, mc // 2, :] = 1.0
    c["g3"] = g3
    c["r3"] = r3
    # GN4 (bn-mode): 8 groups of 8 over 64 ch
    g4 = np.zeros((128, 8), np.float32)
    for k in range(64):
        g4[k, k // 8] = 1.0 / 8.0
    r4 = np.zeros((128, 64), np.float32)
    for m in range(64):
        r4[m // 8, m] = 1.0
    c["g4"] = g4
    c["r4"] = r4
    return c


def _host_weights(w0, b0, w1, b1, w2, b2, w3, b3, weight):
    d = {}
    perm = _perm1()
    # conv0: lhsT[i,o]
    d["w0T"] = np.ascontiguousarray(w0[:, :, 0, 0].T).astype(
        ml_dtypes.bfloat16)  # [256,256]
    d["b0c"] = np.ascontiguousarray(b0.reshape(C, 1)).astype(np.float32)
    # conv1: permuted rows, tap-pair lhsT packs.
    w1p = w1[perm]            # [512, 64, 3, 3]
    b1p = b1[perm]
    # per group g, 5 lhsT [128,128] tensors:
    #   A-pairs dy in {-1,0,1}: rows 0:64 = tap (dy,-1), 64:128 = (dy,0)
    #   B-pair: rows 0:64 = (-1,+1), 64:128 = (0,+1)
    #   single: rows 0:64 = (1,+1), 64:128 = 0
    # A/B input tiles are parity-aware: group g keeps its unshifted x at
    # partition base xb = (g%2)*64 (so the gn2-apply never crosses
    # partitions); the shifted copy lives at the other half, sb = 64-xb.
    w1t = np.zeros((4, 6, 128, 128), np.float32)
    for g in range(4):
        wg = w1p[g * 128:(g + 1) * 128]     # [128 out, 64 in, 3, 3]
        xb = (g % 2) * 64
        sb = 64 - xb
        for i, dy in enumerate((-1, 0, 1)):
            # A-pairs: x-half tap (dy,-1); col-shifted half tap (dy,0)
            w1t[g, i, xb:xb + 64, :] = wg[:, :, dy + 1, 0].T
            w1t[g, i, sb:sb + 64, :] = wg[:, :, dy + 1, 1].T
            # singles: tap (dy,+1) on the x-half only (K=64)
            w1t[g, 3 + i, xb:xb + 64, :] = wg[:, :, dy + 1, 2].T
    d["w1t"] = w1t.astype(ml_dtypes.bfloat16)
    d["b1c"] = np.ascontiguousarray(b1p.reshape(2 * C, 1)).astype(np.float32)
    # conv2: groups=2 (in 128 -> out 32)
    w2t = np.zeros((2, 128, 32), np.float32)
    for g in range(2):
        w2t[g] = w2[g * 32:(g + 1) * 32, :, 0, 0].T
    d["w2t"] = w2t
    d["b2c"] = np.ascontiguousarray(b2.reshape(64, 1)).astype(np.float32)
    # conv3: permuted rows; groups=2 (in 32 -> out 256); K padded to 128.
    w3p = w3[perm]
    w3t = np.zeros((4, 128, 128), np.float32)
    for g in range(4):
        src = w3p[g * 128:(g + 1) * 128, :, 0, 0]      # [128, 32]
        r0 = 0 if g < 2 else 32
        w3t[g, r0:r0 + 32, :] = src.T
    d["w3t"] = w3t
    # final einsum lhsT chunks: row p of chunk g is weight[64g + p%64, :]
    wdT = np.zeros((4, 128, 256), np.float32)
    for g in range(4):
        for p in range(128):
            wdT[g, p] = weight[64 * g + (p % 64)]
    d["wdT"] = wdT.astype(ml_dtypes.bfloat16)
    return d


def _pack_consts(wd, cm):
    fcols = []

    def addf(x):
        x = np.asarray(x, np.float32)
        assert x.shape[0] == 128
        fcols.append(x.reshape(128, -1))

    for c in range(2):
        addf(cm["gm1b"][c]); addf(cm["gm1s"][c]); addf(cm["rep1"][c])
        addf(cm["gm2"][c]); addf(cm["rep2"][c])
    for g in range(4):
        addf(cm["g3"][g]); addf(cm["r3"][g])
    addf(cm["g4"]); addf(cm["r4"])
    b0 = wd["b0c"].reshape(2, 128, 1)
    addf(b0[0]); addf(b0[1])
    b1 = wd["b1c"].reshape(4, 128, 1)
    for g in range(4):
        addf(b1[g])
    b2p = np.zeros((128, 1), np.float32)
    b2p[0:64] = wd["b2c"]
    addf(b2p)
    addf(np.full((128, 1), EPS, np.float32))
    for g in range(2):
        addf(wd["w2t"][g])
    for g in range(4):
        addf(wd["w3t"][g])
    cpack = np.concatenate(fcols, axis=1)

    w0 = np.asarray(wd["w0T"])
    bcols = [w0[0:128], w0[128:256]]
    w1 = np.asarray(wd["w1t"])   # [4, 6, 128, 128]
    for g in range(4):
        for t in range(6):
            bcols.append(w1[g, t])
    wdT = np.asarray(wd["wdT"])
    for g in range(4):
        bcols.append(wdT[g])
    bpack = np.concatenate(bcols, axis=1).astype(ml_dtypes.bfloat16)
    return cpack, bpack


NCF = (32 + 32 + 128 + 16 + 128) * 2 + 4 * (2 + 128) + 8 + 64 \
    + 2 + 4 + 1 + 1 + 2 * 32 + 4 * 128
NBF = 256 * 2 + 4 * 6 * 128 + 4 * 256


# ---------------------------------------------------------------- builder

def build_nc(sim_safe: bool = False):
    nc = bacc.Bacc("TRN2", target_bir_lowering=False, debug=False,
                   num_devices=NCORES)

    def din(name, shape, dt=F32):
        return nc.dram_tensor(name, list(shape), dt, kind="ExternalInput").ap()

    hsb = din("hsb", (BPC, C, H, W), BF16)
    cpack_d = din("cpack", (128, NCF))
    bpack_d = din("bpack", (128, NBF), BF16)
    out_d = nc.dram_tensor("out", [BPC, C, H, W], BF16,
                           kind="ExternalOutput").ap()

    with tile.TileContext(nc) as tc:
        with tc.tile_pool(name="consts", bufs=1) as cst, \
             tc.tile_pool(name="big", bufs=1) as big, \
             tc.tile_pool(name="small", bufs=2) as sm, \
             tc.tile_pool(name="psum", bufs=2, space="PSUM") as psp:

            # ---- load constants / weights (two packed DMAs) ----
            cpk = cst.tile([128, NCF], F32, name="cpk")
            nc.sync.dma_start(out=cpk, in_=cpack_d)
            bpk = cst.tile([128, NBF], BF16, name="bpk")
            nc.sync.dma_start(out=bpk, in_=bpack_d)

            class _Cur:
                def __init__(self):
                    self.o = 0
            _cf, _cb = _Cur(), _Cur()

            def fsl(n):
                s = cpk[:, _cf.o:_cf.o + n]
                _cf.o += n
                return s

            def bsl(n):
                s = bpk[:, _cb.o:_cb.o + n]
                _cb.o += n
                return s

            gm1b_t, gm1s_t, rep1_t, gm2_t, rep2_t = [], [], [], [], []
            for c in range(2):
                gm1b_t.append(fsl(32)); gm1s_t.append(fsl(32))
                rep1_t.append(fsl(128))
                gm2_t.append(fsl(16)); rep2_t.append(fsl(128))
            g3_t, r3_t = [], []
            for g in range(4):
                g3_t.append(fsl(2)); r3_t.append(fsl(128))
            g4_t = fsl(8); r4_t = fsl(64)
            b0_t = [fsl(1) for _ in range(2)]
            b1_t = [fsl(1) for _ in range(4)]
            b2_t = fsl(1)
            eps_t = fsl(1)
            w2_t = [fsl(32) for _ in range(2)]
            w3_t = [fsl(128) for _ in range(4)]
            assert _cf.o == NCF
            w0_t = [bsl(256) for _ in range(2)]
            w1_t = [[bsl(128) for _ in range(6)] for _ in range(4)]
            wd_t = [bsl(256) for _ in range(4)]
            assert _cb.o == NBF

            # shared junk output for tensor_tensor_reduce
            junk = cst.tile([128, 2048], F32, name="ttr_junk")

            # ------------------------------------------------ helpers
            def silu_evac(out_ap, psum_ap, bias_ap, tag, accum=None):
                """out = silu(psum + bias) [+ per-partition row-sum]."""
                acc = accum if USE_ACT_ACCUM else None
                if not sim_safe:
                    nc.scalar.activation(out=out_ap, in_=psum_ap, func=AF.Silu,
                                         bias=bias_ap, scale=1.0,
                                         accum_out=acc)
                    if accum is not None and not USE_ACT_ACCUM:
                        nc.vector.tensor_reduce(out=accum, in_=out_ap,
                                                axis=AX.X, op=ALU.add)
                else:
                    sgf = sm.tile([128, 1024], F32, tag="sg", bufs=1,
                                  name=f"sg_{tag}", uniquify=True)
                    pp = psum_ap.partition_size()
                    ff = psum_ap.free_size()
                    sgt = sgf[0:pp, 0:ff]
                    nc.scalar.activation(out=sgt, in_=psum_ap, func=AF.Sigmoid,
                                         bias=bias_ap, scale=1.0)
                    nc.vector.scalar_tensor_tensor(
                        out=out_ap, in0=psum_ap, scalar=bias_ap, in1=sgt,
                        op0=ALU.add, op1=ALU.mult, accum_out=accum)
                    acc = accum  # sim path: STT accum is fine

            def gn_scale_bias(mvs, gmat_list, rmat_list, ngroups, tag,
                              ncols=2, sums=False):
                """Per-channel (scale, bias[, -mean]) tiles for a group norm.

                mvs: [128,2] tiles per chunk; bn-mode: (mean, var);
                sums-mode: (S1, S2) with 1/(gs*N) baked into gmat.
                """
                nchunk = len(mvs)
                rstats = []
                for ci, mv in enumerate(mvs):
                    if sums:
                        rstats.append(mv)
                        continue
                    r = sm.tile([128, 2], F32, tag=f"r_{tag}", bufs=2 * nchunk)
                    nc.vector.tensor_copy(out=r[:, 0:1], in_=mv[:, 0:1])
                    nc.vector.scalar_tensor_tensor(
                        out=r[:, 1:2], in0=mv[:, 0:1], scalar=mv[:, 0:1],
                        in1=mv[:, 1:2], op0=ALU.mult, op1=ALU.add)
                    rstats.append(r)
                pg = psp.tile([128, 2], F32, tag="gn_ps", bufs=1)
                for ci in range(nchunk):
                    nc.tensor.matmul(pg[0:ngroups, :], gmat_list[ci],
                                     rstats[ci],
                                     start=(ci == 0), stop=(ci == nchunk - 1))
                gt = sm.tile([128, 2], F32, tag=f"gt_{tag}", bufs=2)
                nc.vector.memset(gt, 0.0)
                nc.scalar.copy(out=gt[0:ngroups, :], in_=pg[0:ngroups, :])
                # -var = mean^2 - E[x^2]
                negv = sm.tile([128, 1], F32, tag=f"nv_{tag}", bufs=2)
                nc.vector.scalar_tensor_tensor(
                    out=negv[0:ngroups], in0=gt[0:ngroups, 0:1],
                    scalar=gt[0:ngroups, 0:1], in1=gt[0:ngroups, 1:2],
                    op0=ALU.mult, op1=ALU.subtract)
                sd = sm.tile([128, 1], F32, tag=f"sd_{tag}", bufs=2)
                nc.scalar.activation(out=sd[0:ngroups], in_=negv[0:ngroups],
                                     func=AF.Sqrt, bias=eps_t[0:ngroups],
                                     scale=-1.0)
                rstd = sm.tile([128, 1], F32, tag=f"rs_{tag}", bufs=2)
                nc.vector.reciprocal(out=rstd[0:ngroups], in_=sd[0:ngroups])
                stg = sm.tile([128, 3], F32, tag=f"st_{tag}", bufs=2)
                nc.vector.memset(stg, 0.0)
                nc.vector.tensor_copy(out=stg[0:ngroups, 0:1],
                                      in_=rstd[0:ngroups])
                nc.vector.tensor_scalar(
                    out=stg[0:ngroups, 1:2], in0=gt[0:ngroups, 0:1],
                    scalar1=rstd[0:ngroups], scalar2=-1.0,
                    op0=ALU.mult, op1=ALU.mult)
                if ncols == 3:
                    nc.vector.tensor_scalar(
                        out=stg[0:ngroups, 2:3], in0=gt[0:ngroups, 0:1],
                        scalar1=-1.0, scalar2=None, op0=ALU.mult)
                scs = []
                for ci, rmat in enumerate(rmat_list):
                    mm = rmat.shape[-1]
                    pr = psp.tile([128, 3], F32, tag="gn_ps", bufs=1)
                    nc.tensor.matmul(pr[0:mm, 0:ncols], rmat,
                                     stg[:, 0:ncols], start=True, stop=True)
                    sc = sm.tile([128, 3], F32, tag=f"sc_{tag}",
                                 bufs=2 * nchunk)
                    nc.scalar.copy(out=sc[0:mm, 0:ncols], in_=pr[0:mm, 0:ncols])
                    scs.append(sc)
                return scs

            def sumsq_vector(src, s2_out, tag):
                """S2 = sum(src^2) over 4096 free elems, via 2 chained TTRs."""
                if USE_TTR:
                    h1 = sm.tile([128, 1], F32, tag="sqv", bufs=4,
                                 uniquify=True, name=f"sqv_{tag}")
                    nc.vector.tensor_tensor_reduce(
                        out=junk, in0=src[:, 0:2048], in1=src[:, 0:2048],
                        scale=1.0, scalar=0.0, op0=ALU.mult, op1=ALU.add,
                        accum_out=h1)
                    nc.vector.tensor_tensor_reduce(
                        out=junk, in0=src[:, 2048:4096], in1=src[:, 2048:4096],
                        scale=1.0, scalar=h1, op0=ALU.mult, op1=ALU.add,
                        accum_out=s2_out)
                    return
                parts = sm.tile([128, 2], F32, tag="sqv2", bufs=4,
                                uniquify=True, name=f"sqv2_{tag}")
                jb2 = junk.bitcast(BF16)
                for h in range(2):
                    nc.vector.tensor_tensor(
                        out=jb2[:, 0:2048], in0=src[:, h * 2048:(h + 1) * 2048],
                        in1=src[:, h * 2048:(h + 1) * 2048], op=ALU.mult)
                    nc.vector.tensor_reduce(out=parts[:, h:h + 1],
                                            in_=jb2[:, 0:2048],
                                            axis=AX.X, op=ALU.add)
                nc.vector.tensor_reduce(out=s2_out, in_=parts,
                                        axis=AX.X, op=ALU.add)

            def sumsq_scalar(src, s2_out, tag):
                """S2 via 2 scalar Square activations with accumulators."""
                if not USE_ACT_ACCUM:
                    sumsq_vector(src, s2_out, tag)
                    return
                parts = sm.tile([128, 2], F32, tag="sqp", bufs=4,
                                uniquify=True, name=f"sqs_{tag}")
                jb = junk.bitcast(BF16)
                for h in range(2):
                    nc.scalar.activation(
                        out=jb[:, 0:2048], in_=src[:, h * 2048:(h + 1) * 2048],
                        func=AF.Square, scale=1.0,
                        accum_out=parts[:, h:h + 1])
                nc.vector.tensor_reduce(out=s2_out, in_=parts,
                                        axis=AX.X, op=ALU.add)

            # ------------------------------------------------ per-sample state
            st = [dict() for _ in range(BPC)]

            def phase_load(b):
                s = st[b]
                hsv = hsb[b].rearrange("c h w -> c (h w)")
                s["xw"] = [big.tile([128, NPIX], BF16, tag="xw", bufs=2,
                                    name=f"xw{b}_{i}") for i in range(2)]
                for c in range(2):
                    for qd in range(4):
                        ql = bass.ts(qd, 1024)
                        nc.sync.dma_start(
                            out=s["xw"][c][:, ql],
                            in_=hsv[c * 128:(c + 1) * 128, ql])

            def phase_gn1(b):
                s = st[b]
                bst1 = [sm.tile([128, NT, 6], F32, tag="bst1", bufs=4,
                                name=f"bst1_{b}_{i}") for i in range(2)]
                for c in range(2):
                    for n in range(NT):
                        nc.vector.bn_stats(out=bst1[c][:, n, :],
                                           in_=s["xw"][c][:, bass.ts(n, 512)])
                mv1 = []
                for c in range(2):
                    mv = sm.tile([128, 2], F32, tag="mv1", bufs=4,
                                 name=f"mv1_{b}_{c}")
                    nc.vector.bn_aggr(out=mv, in_=bst1[c])
                    mv1.append(mv)
                s["mv1"] = mv1

            def phase_w0fold(b):
                s = st[b]
                sc1 = gn_scale_bias(s["mv1"], gm1b_t, rep1_t, 32, "gn1")
                w0s = [sm.tile([128, 256], BF16, tag="w0s", bufs=4,
                               name=f"w0s{b}_{i}") for i in range(2)]
                t1b = [sm.tile([128, 1], BF16, tag="t1b", bufs=4,
                               name=f"t1b{b}_{i}") for i in range(2)]
                for c in range(2):
                    nc.vector.tensor_scalar_mul(out=w0s[c], in0=w0_t[c],
                                                scalar1=sc1[c][:, 0:1])
                    nc.vector.tensor_copy(out=t1b[c], in_=sc1[c][:, 1:2])
                b0p = [sm.tile([128, 1], F32, tag="b0p", bufs=4,
                               name=f"b0p{b}_{i}") for i in range(2)]
                for m in range(2):
                    pb = psp.tile([128, 1], F32, tag="gn_ps", bufs=1)
                    for kc in range(2):
                        nc.tensor.matmul(
                            pb, w0s[kc][:, m * 128:(m + 1) * 128], t1b[kc],
                            start=(kc == 0), stop=(kc == 1))
                    nc.scalar.activation(out=b0p[m], in_=pb,
                                         func=AF.Identity, bias=b0_t[m],
                                         scale=1.0)
                s["w0s"], s["b0p"] = w0s, b0p

            def phase_conv0(b):
                s = st[b]
                y0 = [big.tile([128, NPIX], BF16, tag="y0", bufs=2,
                               name=f"y0{b}_{i}") for i in range(2)]
                acc0 = [sm.tile([128, 4], F32, tag="acc0", bufs=4,
                                name=f"acc0_{b}_{i}") for i in range(2)]
                for m in range(2):
                    for q in range(4):   # 1024-pix strips
                        pt = psp.tile([128, 1024], F32, tag="acc", bufs=3,
                                      name=f"pc0_{b}_{m}_{q}", uniquify=True)
                        for hh in range(2):
                            n = q * 2 + hh
                            for kc in range(2):
                                nc.tensor.matmul(
                                    pt[:, hh * 512:(hh + 1) * 512],
                                    s["w0s"][kc][:, m * 128:(m + 1) * 128],
                                    s["xw"][kc][:, bass.ts(n, 512)],
                                    start=(kc == 0), stop=(kc == 1))
                        silu_evac(y0[m][:, q * 1024:(q + 1) * 1024], pt,
                                  s["b0p"][m], f"c0_{b}",
                                  accum=acc0[m][:, q:q + 1])
                s["y0"], s["acc0"] = y0, acc0

            def phase_gn2(b):
                s = st[b]
                mv2 = []
                for c in range(2):
                    mv = sm.tile([128, 2], F32, tag="mv2", bufs=4,
                                 name=f"mv2_{b}_{c}")
                    nc.vector.tensor_reduce(out=mv[:, 0:1], in_=s["acc0"][c],
                                            axis=AX.X, op=ALU.add)
                    sumsq_scalar(s["y0"][c], mv[:, 1:2], f"g2_{b}_{c}")
                    mv2.append(mv)
                s["sc2"] = gn_scale_bias(mv2, gm2_t, rep2_t, 16, "gn2",
                                         sums=True)

            def phase_xp(b):
                """Build per-group padded input tiles A (x | x-shift-dx+1)
                and B (x | x-shift-dy+1)."""
                s = st[b]
                At = [big.tile([128, PADW, PADW], BF16, tag="Atile", bufs=4,
                               name=f"A{b}_{g}") for g in range(4)]
                for g in range(4):
                    a = At[g]
                    # group g's unshifted x lives at partition base xb so the
                    # gn2-apply (vector: no cross-partition path) stays
                    # base-aligned with y0; shifted copies move via DMA.
                    xb = (g % 2) * 64
                    sb = 64 - xb
                    ax = a[xb:xb + 64]
                    # zero borders of the x half (rows 0,65; cols 0,65)
                    nc.gpsimd.memset(ax[:, 0:1, :], 0.0)
                    nc.gpsimd.memset(ax[:, PADW - 1:PADW, :], 0.0)
                    nc.gpsimd.memset(ax[:, 1:PADW - 1, 0:1], 0.0)
                    nc.gpsimd.memset(ax[:, 1:PADW - 1, PADW - 1:PADW], 0.0)
                    # gn2 apply: y0 rows of this group's in-channels
                    src = s["y0"][g // 2]
                    sc2 = s["sc2"][g // 2]
                    nc.vector.tensor_scalar(
                        out=ax[:, 1:H + 1, 1:W + 1],
                        in0=src[xb:xb + 64, :].rearrange(
                            "p (h w) -> p h w", h=H),
                        scalar1=sc2[xb:xb + 64, 0:1],
                        scalar2=sc2[xb:xb + 64, 1:2],
                        op0=ALU.mult, op1=ALU.add)
                    # A shifted half: x shifted one col left (DMA cross-move)
                    nc.gpsimd.memset(a[sb:sb + 64, :, PADW - 1:PADW], 0.0)
                    nc.sync.dma_start(out=a[sb:sb + 64, :, 0:PADW - 1],
                                      in_=ax[:, :, 1:PADW])
                s["A"] = At

            def phase_conv1(b):
                s = st[b]
                y1 = [big.tile([128, NPIX], BF16, tag="y1", bufs=8,
                               name=f"y1{b}_{g}") for g in range(4)]
                acc1 = [sm.tile([128, 4], F32, tag="acc1", bufs=4,
                                name=f"acc1_{b}_{g}") for g in range(4)]
                pooled = sm.tile([128, 4, Hn * Wn], BF16, tag="pooled",
                                 bufs=2, name=f"pool{b}")
                mv3 = [sm.tile([128, 2], F32, tag="mv3", bufs=4,
                               name=f"mv3_{b}_{g}") for g in range(4)]
                for g in range(4):
                    a = s["A"][g]
                    for q in range(4):     # 1024-pix strips (16 rows)
                        pt = psp.tile([128, 1024], F32, tag="acc", bufs=3,
                                      name=f"pc1_{b}_{g}_{q}", uniquify=True)
                        for hh in range(2):
                            r0 = (q * 2 + hh) * WS
                            po = pt[:, hh * 512:(hh + 1) * 512]
                            for i, dy in enumerate((-1, 0, 1)):
                                nc.tensor.matmul(
                                    po, w1_t[g][3 + i],
                                    a[:, r0 + dy + 1:r0 + dy + 9, 2:W + 2],
                                    start=(i == 0), stop=False)
                            for i, dy in enumerate((-1, 0, 1)):
                                nc.tensor.matmul(
                                    po, w1_t[g][i],
                                    a[:, r0 + dy + 1:r0 + dy + 9, 0:W],
                                    start=False, stop=(i == 2))
                        silu_evac(y1[g][:, q * 1024:(q + 1) * 1024], pt,
                                  b1_t[g], f"c1_{b}",
                                  accum=acc1[g][:, q:q + 1])
                    # window pooling: tree adds (2-byte 2x mode), partials
                    # live in bf16 regions of the shared junk scratch
                    yv = y1[g].rearrange("p (a ws) -> p a ws", ws=WS)
                    jb = junk.bitcast(BF16)
                    t1 = jb[:, 0:2048].rearrange("p (a b) -> p a b", b=4)
                    nc.vector.tensor_tensor(out=t1, in0=yv[:, :, 0:4],
                                            in1=yv[:, :, 4:8], op=ALU.add)
                    t2 = jb[:, 2048:3072].rearrange("p (a b) -> p a b", b=2)
                    nc.vector.tensor_tensor(out=t2, in0=t1[:, :, 0:2],
                                            in1=t1[:, :, 2:4], op=ALU.add)
                    pa = jb[:, 3072:3584]
                    nc.vector.tensor_tensor(out=pa.rearrange("p (h wn) -> p h wn", wn=Wn),
                                            in0=t2[:, :, 0:1].squeeze(2).rearrange("p (h wn) -> p h wn", wn=Wn),
                                            in1=t2[:, :, 1:2].squeeze(2).rearrange("p (h wn) -> p h wn", wn=Wn),
                                            op=ALU.add)
                    # h-direction tree over h2: keep 3D by slicing the
                    # contiguous (h2 wn) tail of each hn row
                    pv = pa.rearrange("p (hn x) -> p hn x", hn=Hn)  # x=h2*wn
                    u1 = jb[:, 3584:3840].rearrange("p (hn x) -> p hn x",
                                                    hn=Hn)          # x=32
                    nc.vector.tensor_tensor(out=u1, in0=pv[:, :, 0:32],
                                            in1=pv[:, :, 32:64], op=ALU.add)
                    u2 = jb[:, 3840:3968].rearrange("p (hn x) -> p hn x",
                                                    hn=Hn)          # x=16
                    nc.vector.tensor_tensor(out=u2, in0=u1[:, :, 0:16],
                                            in1=u1[:, :, 16:32], op=ALU.add)
                    nc.vector.tensor_tensor(
                        out=pooled[:, g, :].rearrange("p (a b) -> p a b",
                                                      b=Wn),
                        in0=u2[:, :, 0:8], in1=u2[:, :, 8:16], op=ALU.add)
                    # GN3 stats
                    nc.vector.tensor_reduce(out=mv3[g][:, 0:1], in_=acc1[g],
                                            axis=AX.X, op=ALU.add)
                    sumsq_scalar(y1[g], mv3[g][:, 1:2], f"g3_{b}_{g}")
                s["y1"], s["pooled"], s["mv3"] = y1, pooled, mv3

            def phase_gn3(b):
                s = st[b]
                s["sc3"] = gn_scale_bias(s["mv3"], g3_t, r3_t, 2, "gn3",
                                         ncols=3, sums=True)

            def phase_attn(b):
                s = st[b]
                sc3 = s["sc3"]
                pooled = s["pooled"]
                # Radix amax: partners sit in opposite partition halves, so
                # stage the other half across with DMA (engines cannot move
                # data across partitions), then max at matching base.
                # am[0] rows = c 0:128 (groups 0,1), am[1] = c 128:256.
                am = [sm.tile([128, 64], F32, tag="am", bufs=2,
                              name=f"am{b}_{i}") for i in range(2)]
                stage = sm.tile([128, 2, 64], BF16, tag="pstage", bufs=2,
                                name=f"pstage_{b}")
                # r1 halves of even chunks -> base 0 (cols 0,1 = g 0,2)
                pv4 = pooled.rearrange("p (i j) w -> p i j w", i=2)
                nc.sync.dma_start(out=stage[0:64],
                                  in_=pv4[64:128, :, 0, :])
                # r0 halves of odd chunks -> base 64 (cols 0,1 = g 1,3)
                nc.sync.dma_start(out=stage[64:128],
                                  in_=pv4[0:64, :, 1, :])
                for g in range(4):
                    xb = (g % 2) * 64
                    pg = pooled[xb:xb + 64, g, :]
                    dst = am[g // 2][xb:xb + 64, :]
                    nc.vector.tensor_tensor(out=dst, in0=pg,
                                            in1=stage[xb:xb + 64, g // 2, :],
                                            op=ALU.max)
                    # normalize: am*(rstd3/64) + t3  (scalars uniform in group)
                    s64 = sm.tile([128, 1], F32, tag="s64", bufs=2,
                                  name=f"s64_{b}_{g}", uniquify=True)
                    nc.vector.tensor_scalar(
                        out=s64[xb:xb + 64], in0=sc3[g][xb:xb + 64, 0:1],
                        scalar1=1.0 / (WS * WS), scalar2=None, op0=ALU.mult)
                    nc.vector.tensor_scalar(
                        out=dst, in0=dst, scalar1=s64[xb:xb + 64],
                        scalar2=sc3[g][xb:xb + 64, 1:2],
                        op0=ALU.mult, op1=ALU.add)
                # conv2 (1x1 g=2, 256->64) + silu
                p2 = psp.tile([128, 64], F32, tag="tp", bufs=1)
                for g in range(2):
                    nc.tensor.matmul(p2[g * 32:(g + 1) * 32, :], w2_t[g],
                                     am[g], start=True, stop=True)
                a2 = sm.tile([128, 64], F32, tag="a2", bufs=2)
                nc.vector.memset(a2, 0.0)
                silu_evac(a2[0:64, :], p2[0:64, :], b2_t[0:64], f"c2_{b}")
                # GN4
                mv4pad = sm.tile([128, 2], F32, tag="mv4", bufs=2)
                nc.vector.memset(mv4pad, 0.0)
                bst4 = sm.tile([128, 1, 6], F32, tag="bst4", bufs=2)
                nc.vector.bn_stats(out=bst4[0:64], in_=a2[0:64].unsqueeze(1))
                nc.vector.bn_aggr(out=mv4pad[0:64], in_=bst4[0:64])
                sc4 = gn_scale_bias([mv4pad], [g4_t], [r4_t], 8, "gn4")[0]
                a2n = sm.tile([128, 64], F32, tag="a2n", bufs=2)
                nc.vector.memset(a2n, 0.0)
                nc.vector.tensor_scalar(
                    out=a2n[0:64], in0=a2[0:64],
                    scalar1=sc4[0:64, 0:1], scalar2=sc4[0:64, 1:2],
                    op0=ALU.mult, op1=ALU.add)
                # conv3 (1x1 g=2, 64->512), b3 = 0; split each chunk's
                # matmul into its two radix halves (M=64 each) so the
                # subtract happens at matching partition base 0.
                dall = sm.tile([64, 4, 64], F32, tag="dall", bufs=1)
                for g in range(4):
                    p3 = psp.tile([64, 128], F32, tag="tp", bufs=1,
                                  name=f"p3_{b}_{g}", uniquify=True)
                    nc.tensor.matmul(p3[:, 0:64], w3_t[g][:, 0:64], a2n,
                                     start=True, stop=True)
                    nc.tensor.matmul(p3[:, 64:128], w3_t[g][:, 64:128], a2n,
                                     start=True, stop=True)
                    a3b = sm.tile([64, 64], F32, tag="a3b", bufs=2,
                                  name=f"a3b_{b}_{g}", uniquify=True)
                    nc.scalar.copy(out=a3b, in_=p3[:, 64:128])
                    nc.vector.tensor_tensor(out=dall[:, g, :],
                                            in0=p3[:, 0:64],
                                            in1=a3b,
                                            op=ALU.subtract)
                # sigma(+d) at base 0; DMA d up to base 64 for sigma(-d)
                sintP = sm.tile([64, 4, 64], BF16, tag="sintP", bufs=2,
                                name=f"sintP{b}")
                nc.scalar.activation(out=sintP, in_=dall,
                                     func=AF.Sigmoid, scale=1.0)
                dhi = sm.tile([128, 4, 64], F32, tag="dhi", bufs=2,
                              name=f"dhi{b}")
                nc.sync.dma_start(out=dhi[64:128], in_=dall)
                sintN = sm.tile([128, 4, 64], BF16, tag="sintN", bufs=2,
                                name=f"sintN{b}")
                nc.scalar.activation(out=sintN[64:128], in_=dhi[64:128],
                                     func=AF.Sigmoid, scale=-1.0)
                # expand gate over ws on gpsimd (3D, per partition half)
                gx = [sm.tile([128, Hn * Wn, WS], BF16, tag="gx", bufs=4,
                              name=f"gx{b}_{g}") for g in range(4)]
                for g in range(4):
                    nc.vector.tensor_copy(
                        out=gx[g][0:64],
                        in_=sintP[:, g, :].unsqueeze(2).broadcast_to(
                            [64, Hn * Wn, WS]))
                    nc.vector.tensor_copy(
                        out=gx[g][64:128],
                        in_=sintN[64:128, g, :].unsqueeze(2).broadcast_to(
                            [64, Hn * Wn, WS]))
                s["gx"] = gx

            def phase_gate(b):
                s = st[b]
                # fold GN3 rstd into final matmul weights
                wds = [sm.tile([128, 256], BF16, tag="wds", bufs=4,
                               name=f"wds{b}_{g}") for g in range(4)]
                for g in range(4):
                    nc.vector.tensor_scalar_mul(
                        out=wds[g], in0=wd_t[g], scalar1=s["sc3"][g][:, 0:1])
                # z = (y1 + (-mean3)) * gate, OUT-OF-PLACE into recycled
                # A-tile slots (in-place DVE ops appear to forfeit the
                # 2-byte fast modes; the A tiles are dead by gate time so
                # this costs no SBUF). Iterate h2 so the gate operand is
                # the plain packed gx tile with no broadcast.
                zt = [big.tile([128, NPIX], BF16, tag="Atile", bufs=4,
                               name=f"z{b}_{g}", uniquify=True)
                      for g in range(4)]
                for g in range(4):
                    yv = s["y1"][g].rearrange(
                        "p (hn h2 x) -> p hn h2 x", hn=Hn, h2=WS)
                    zv = zt[g].rearrange(
                        "p (hn h2 x) -> p hn h2 x", hn=Hn, h2=WS)
                    gv = s["gx"][g].rearrange(
                        "p (hn wn) ws -> p hn (wn ws)", hn=Hn)
                    for h2 in range(WS):
                        nc.vector.scalar_tensor_tensor(
                            out=zv[:, :, h2, :], in0=yv[:, :, h2, :],
                            scalar=s["sc3"][g][:, 2:3],
                            in1=gv, op0=ALU.add, op1=ALU.mult)
                s["wds"], s["z"] = wds, zt

            def phase_final(b):
                s = st[b]
                ot = [big.tile([128, NPIX], BF16, tag="ot", bufs=2,
                               name=f"ot{b}_{i}") for i in range(2)]
                acc5 = [sm.tile([128, 4], F32, tag="acc5", bufs=4,
                                name=f"acc5_{b}_{i}") for i in range(2)]
                mv5 = [sm.tile([128, 2], F32, tag="mv5", bufs=4,
                               name=f"mv5f_{b}_{i}") for i in range(2)]
                for m in range(2):
                    for q in range(4):
                        pt = psp.tile([128, 1024], F32, tag="acc", bufs=3,
                                      name=f"pcf_{b}_{m}_{q}", uniquify=True)
                        for hh in range(2):
                            n = q * 2 + hh
                            for kc in range(4):
                                nc.tensor.matmul(
                                    pt[:, hh * 512:(hh + 1) * 512],
                                    s["wds"][kc][:, m * 128:(m + 1) * 128],
                                    s["z"][kc][:, bass.ts(n, 512)],
                                    start=(kc == 0), stop=(kc == 3))
                        if USE_ACT_ACCUM:
                            nc.scalar.activation(
                                out=ot[m][:, q * 1024:(q + 1) * 1024], in_=pt,
                                func=AF.Copy, accum_out=acc5[m][:, q:q + 1])
                        else:
                            nc.scalar.copy(
                                out=ot[m][:, q * 1024:(q + 1) * 1024], in_=pt)
                            nc.vector.tensor_reduce(
                                out=acc5[m][:, q:q + 1],
                                in_=ot[m][:, q * 1024:(q + 1) * 1024],
                                axis=AX.X, op=ALU.add)
                    # chunk stats immediately: the squares of chunk m
                    # overlap the next chunk's matmuls
                    nc.vector.tensor_reduce(out=mv5[m][:, 0:1],
                                            in_=acc5[m],
                                            axis=AX.X, op=ALU.add)
                    sumsq_scalar(ot[m], mv5[m][:, 1:2], f"g5_{b}_{m}")
                s["ot"], s["acc5"], s["mv5"] = ot, acc5, mv5

            def phase_gn5(b):
                s = st[b]
                s["sc5"] = gn_scale_bias(s["mv5"], gm1s_t, rep1_t, 32, "gn5",
                                         sums=True)

            def phase_out(b):
                s = st[b]
                ov = out_d[b].rearrange("c h w -> c (h w)")
                hsv = hsb[b].rearrange("c h w -> c (h w)")
                for c in range(2):
                    sc5 = s["sc5"][c]
                    for h2 in range(2):
                        sl = bass.ts(h2, 2048)
                        xr = sm.tile([128, 2048], BF16, tag="xr", bufs=2,
                                     name=f"xr{b}_{c}_{h2}", uniquify=True)
                        nc.sync.dma_start(
                            out=xr, in_=hsv[c * 128:(c + 1) * 128, sl])
                        ts = s["ot"][c][:, sl]
                        nc.vector.tensor_scalar(
                            out=ts, in0=ts, scalar1=sc5[:, 0:1],
                            scalar2=sc5[:, 1:2], op0=ALU.mult, op1=ALU.add)
                        nc.vector.tensor_tensor(
                            out=ts, in0=ts, in1=xr, op=ALU.add)
                        nc.sync.dma_start(
                            out=ov[c * 128:(c + 1) * 128, sl], in_=ts)

            def scoped(name, fn, *a):
                _s, _ = nc.enter_named_scope(name, False)
                fn(*a)
                nc.leave_named_scope(name, _s, False)

            # ---------------- interleaved schedule ----------------
            scoped("ld0", phase_load, 0)
            scoped("gn1_0", phase_gn1, 0)
            scoped("w0f_0", phase_w0fold, 0)
            scoped("ld1", phase_load, 1)
            scoped("conv0_0", phase_conv0, 0)
            scoped("gn1_1", phase_gn1, 1)
            scoped("gn2_0", phase_gn2, 0)
            scoped("xp_0", phase_xp, 0)
            scoped("conv1_0", phase_conv1, 0)
            scoped("w0f_1", phase_w0fold, 1)
            scoped("conv0_1", phase_conv0, 1)
            scoped("gn3_0", phase_gn3, 0)
            scoped("attn_0", phase_attn, 0)
            scoped("gn2_1", phase_gn2, 1)
            scoped("xp_1", phase_xp, 1)
            scoped("conv1_1", phase_conv1, 1)
            scoped("gate_0", phase_gate, 0)
            scoped("gn3_1", phase_gn3, 1)
            scoped("attn_1", phase_attn, 1)
            scoped("final_0", phase_final, 0)
            scoped("gn5_0", phase_gn5, 0)
            scoped("gate_1", phase_gate, 1)
            scoped("out_0", phase_out, 0)
            scoped("final_1", phase_final, 1)
            scoped("gn5_1", phase_gn5, 1)
            scoped("out_1", phase_out, 1)

    nc.compile()
    return nc


# ---------------------------------------------------------------- entry

_CACHE = {}


def _get_nc(sim_safe=False):
    key = bool(sim_safe)
    if key not in _CACHE:
        _CACHE[key] = build_nc(sim_safe=key)
    return _CACHE[key]


def make_in_maps(inputs):
    hs_full = np.ascontiguousarray(inputs["hidden_state"], dtype=np.float32)
    wd = _host_weights(
        np.asarray(inputs["w0"], np.float32), np.asarray(inputs["b0"], np.float32),
        np.asarray(inputs["w1"], np.float32), np.asarray(inputs["b1"], np.float32),
        np.asarray(inputs["w2"], np.float32), np.asarray(inputs["b2"], np.float32),
        np.asarray(inputs["w3"], np.float32), np.asarray(inputs["b3"], np.float32),
        np.asarray(inputs["weight"], np.float32))
    cm = _host_consts()
    cpack, bpack = _pack_consts(wd, cm)
    assert cpack.shape[1] == NCF, (cpack.shape, NCF)
    assert bpack.shape[1] == NBF, (bpack.shape, NBF)
    shared = {"cpack": cpack, "bpack": bpack}
    in_maps = []
    for i in range(NCORES):
        m = dict(shared)
        m["hsb"] = np.ascontiguousarray(
            hs_full[i * BPC:(i + 1) * BPC]).astype(ml_dtypes.bfloat16)
        in_maps.append(m)
    return in_maps


def kernel(**inputs):
    from concourse import bass_utils
    nc = _get_nc(sim_safe=False)
    in_maps = make_in_maps(inputs)
    res = bass_utils.run_bass_kernel_spmd(nc, in_maps,
                                          core_ids=list(range(NCORES)))
    out = np.concatenate([res.results[i]["out"] for i in range(NCORES)],
                         axis=0)
    return out.astype(np.float32)


# revision 51
# speedup vs baseline: 1.1879x; 1.1879x over previous
"""Trainium2 Bass kernel for nn_Block_16544214024520 (dense_cnn).

Data-parallel over batch: 16 samples -> 2 per NeuronCore x 8 cores.
All parameters replicated. Per-sample layout: channels on partitions
(256 = 2 chunks of 128), pixels (64x64 = 4096) on the free dim.

Reference pipeline (per sample):
  gn(32) -> 1x1 conv(256->256)+silu -> gn(16) -> 3x3 grouped conv
  (g=4, 256->512)+silu -> gn(2) -> window-mean(8x8) -> radix amax ->
  1x1 g-conv(256->64)+silu -> gn(8) -> 1x1 g-conv(64->512) ->
  softmax over radix(2) -> gated combine -> channel matmul(256->256)
  -> gn(32) -> +residual

v2 design notes:
  - conv1 out-channels are permuted within each group to
    [r=0 c-block | r=1 c-block] so the radix amax / softmax are
    partition-offset ops (no PE transposes anywhere).
  - conv1 runs as 4 tap-pair matmuls (K=128) + 1 single (K=64 padded)
    per strip, using per-group input tiles that hold [x | x-shifted]
    on the partition dim (shifted copies built by DMA + vector).
  - group-norm stats: means ride the activation accumulators of the
    psum evacuations; sum-of-squares via scalar Square acts or vector
    tensor_tensor_reduce, balanced across engines.
  - gating is one 4x-mode scalar_tensor_tensor per (group, sample).
  - bf16 input only (residual add in bf16), bf16 DRAM output
    (host casts to fp32).
  - the two samples are interleaved so the attn latency chain of one
    overlaps the conv matmuls of the other.
"""

import os
import sys

for _p in ("/opt/trn_rl_repo", "/opt/pypackages"):
    if _p not in sys.path:
        sys.path.append(_p)

import ml_dtypes
import numpy as np

import concourse.bass as bass  # noqa: F401
import concourse.mybir as mybir
import concourse.tile as tile
from concourse import bacc

F32 = mybir.dt.float32
BF16 = mybir.dt.bfloat16
AF = mybir.ActivationFunctionType
ALU = mybir.AluOpType
AX = mybir.AxisListType

NCORES = 8
BPC = 2          # samples per core
C = 256          # channels
H = W = 64
NPIX = H * W     # 4096
PADW = W + 2     # 66
Hn = Wn = 8      # window grid
WS = 8           # window size
EPS = 1e-5
NT = 8           # n-tiles of 512 pixels
USE_SBUF_DMA = False
USE_ACT_ACCUM = True
USE_TTR = False


def _perm1():
    """conv1/conv3 out-channel permutation: within each 128-row group
    chunk g, rows [0:64] = (c=64g+p, r=0), rows [64:128] = (c, r=1).
    Original channel of (c, r) is 2c + r."""
    p = np.zeros(512, np.int64)
    for g in range(4):
        for q in range(64):
            p[128 * g + q] = 2 * (64 * g + q)          # r = 0
            p[128 * g + 64 + q] = 2 * (64 * g + q) + 1  # r = 1
    return p


# ---------------------------------------------------------------- host prep

def _host_consts():
    c = {}
    # GN1 (bn-mode): 32 groups of 8 over 256 ch
    gm1b = np.zeros((2, 128, 32), np.float32)
    rep1 = np.zeros((2, 128, 128), np.float32)
    for ch in range(2):
        for k in range(128):
            g = (128 * ch + k) // 8
            gm1b[ch, k, g] = 1.0 / 8.0
        for m in range(128):
            rep1[ch, (128 * ch + m) // 8 % 128, m] = 1.0
    c["gm1b"] = gm1b
    c["gm1s"] = gm1b / NPIX     # GN5 (sums-mode)
    c["rep1"] = rep1
    # GN2 (sums-mode): 16 groups of 16 over 256 ch
    gm2 = np.zeros((2, 128, 16), np.float32)
    rep2 = np.zeros((2, 128, 128), np.float32)
    for ch in range(2):
        for k in range(128):
            gm2[ch, k, (128 * ch + k) // 16] = 1.0 / (16.0 * NPIX)
        for m in range(128):
            rep2[ch, (128 * ch + m) // 16, m] = 1.0
    c["gm2"] = gm2
    c["rep2"] = rep2
    # GN3 (sums-mode): 2 groups of 256 over 512 ch; chunks 0,1 -> g0
    # (permutation within chunks doesn't change group membership)
    g3 = np.zeros((4, 128, 2), np.float32)
    r3 = np.zeros((4, 128, 128), np.float32)
    for mc in range(4):
        g3[mc, :, mc // 2] = 1.0 / (256.0 * NPIX)
        r3[mc, mc // 2, :] = 1.0
    c["g3"] = g3
    c["r3"] = r3
    # GN4 (bn-mode): 8 groups of 8 over 64 ch
    g4 = np.zeros((128, 8), np.float32)
    for k in range(64):
        g4[k, k // 8] = 1.0 / 8.0
    r4 = np.zeros((128, 64), np.float32)
    for m in range(64):
        r4[m // 8, m] = 1.0
    c["g4"] = g4
    c["r4"] = r4
    return c


def _host_weights(w0, b0, w1, b1, w2, b2, w3, b3, weight):
    d = {}
    perm = _perm1()
    # conv0: lhsT[i,o]
    d["w0T"] = np.ascontiguousarray(w0[:, :, 0, 0].T).astype(
        ml_dtypes.bfloat16)  # [256,256]
    d["b0c"] = np.ascontiguousarray(b0.reshape(C, 1)).astype(np.float32)
    # conv1: permuted rows, tap-pair lhsT packs.
    w1p = w1[perm]            # [512, 64, 3, 3]
    b1p = b1[perm]
    # per group g, 5 lhsT [128,128] tensors:
    #   A-pairs dy in {-1,0,1}: rows 0:64 = tap (dy,-1), 64:128 = (dy,0)
    #   B-pair: rows 0:64 = (-1,+1), 64:128 = (0,+1)
    #   single: rows 0:64 = (1,+1), 64:128 = 0
    # A/B input tiles are parity-aware: group g keeps its unshifted x at
    # partition base xb = (g%2)*64 (so the gn2-apply never crosses
    # partitions); the shifted copy lives at the other half, sb = 64-xb.
    w1t = np.zeros((4, 6, 128, 128), np.float32)
    for g in range(4):
        wg = w1p[g * 128:(g + 1) * 128]     # [128 out, 64 in, 3, 3]
        xb = (g % 2) * 64
        sb = 64 - xb
        for i, dy in enumerate((-1, 0, 1)):
            # A-pairs: x-half tap (dy,-1); col-shifted half tap (dy,0)
            w1t[g, i, xb:xb + 64, :] = wg[:, :, dy + 1, 0].T
            w1t[g, i, sb:sb + 64, :] = wg[:, :, dy + 1, 1].T
            # singles: tap (dy,+1) on the x-half only (K=64)
            w1t[g, 3 + i, xb:xb + 64, :] = wg[:, :, dy + 1, 2].T
    d["w1t"] = w1t.astype(ml_dtypes.bfloat16)
    d["b1c"] = np.ascontiguousarray(b1p.reshape(2 * C, 1)).astype(np.float32)
    # conv2: groups=2 (in 128 -> out 32)
    w2t = np.zeros((2, 128, 32), np.float32)
    for g in range(2):
        w2t[g] = w2[g * 32:(g + 1) * 32, :, 0, 0].T
    d["w2t"] = w2t
    d["b2c"] = np.ascontiguousarray(b2.reshape(64, 1)).astype(np.float32)
    # conv3: permuted rows; groups=2 (in 32 -> out 256); K padded to 128.
    w3p = w3[perm]
    w3t = np.zeros((4, 128, 128), np.float32)
    for g in range(4):
        src = w3p[g * 128:(g + 1) * 128, :, 0, 0]      # [128, 32]
        r0 = 0 if g < 2 else 32
        w3t[g, r0:r0 + 32, :] = src.T
    d["w3t"] = w3t
    # final einsum lhsT chunks: row p of chunk g is weight[64g + p%64, :]
    wdT = np.zeros((4, 128, 256), np.float32)
    for g in range(4):
        for p in range(128):
            wdT[g, p] = weight[64 * g + (p % 64)]
    d["wdT"] = wdT.astype(ml_dtypes.bfloat16)
    return d


def _pack_consts(wd, cm):
    fcols = []

    def addf(x):
        x = np.asarray(x, np.float32)
        assert x.shape[0] == 128
        fcols.append(x.reshape(128, -1))

    for c in range(2):
        addf(cm["gm1b"][c]); addf(cm["gm1s"][c]); addf(cm["rep1"][c])
        addf(cm["gm2"][c]); addf(cm["rep2"][c])
    for g in range(4):
        addf(cm["g3"][g]); addf(cm["r3"][g])
    addf(cm["g4"]); addf(cm["r4"])
    b0 = wd["b0c"].reshape(2, 128, 1)
    addf(b0[0]); addf(b0[1])
    b1 = wd["b1c"].reshape(4, 128, 1)
    for g in range(4):
        addf(b1[g])
    b2p = np.zeros((128, 1), np.float32)
    b2p[0:64] = wd["b2c"]
    addf(b2p)
    addf(np.full((128, 1), EPS, np.float32))
    for g in range(2):
        addf(wd["w2t"][g])
    for g in range(4):
        addf(wd["w3t"][g])
    cpack = np.concatenate(fcols, axis=1)

    w0 = np.asarray(wd["w0T"])
    bcols = [w0[0:128], w0[128:256]]
    w1 = np.asarray(wd["w1t"])   # [4, 6, 128, 128]
    for g in range(4):
        for t in range(6):
            bcols.append(w1[g, t])
    wdT = np.asarray(wd["wdT"])
    for g in range(4):
        bcols.append(wdT[g])
    bpack = np.concatenate(bcols, axis=1).astype(ml_dtypes.bfloat16)
    return cpack, bpack


NCF = (32 + 32 + 128 + 16 + 128) * 2 + 4 * (2 + 128) + 8 + 64 \
    + 2 + 4 + 1 + 1 + 2 * 32 + 4 * 128
NBF = 256 * 2 + 4 * 6 * 128 + 4 * 256


# ---------------------------------------------------------------- builder

def build_nc(sim_safe: bool = False):
    nc = bacc.Bacc("TRN2", target_bir_lowering=False, debug=False,
                   num_devices=NCORES)

    def din(name, shape, dt=F32):
        return nc.dram_tensor(name, list(shape), dt, kind="ExternalInput").ap()

    hsb = din("hsb", (BPC, C, H, W), BF16)
    cpack_d = din("cpack", (128, NCF))
    bpack_d = din("bpack", (128, NBF), BF16)
    out_d = nc.dram_tensor("out", [BPC, C, H, W], BF16,
                           kind="ExternalOutput").ap()

    with tile.TileContext(nc) as tc:
        with tc.tile_pool(name="consts", bufs=1) as cst, \
             tc.tile_pool(name="big", bufs=1) as big, \
             tc.tile_pool(name="small", bufs=2) as sm, \
             tc.tile_pool(name="psum", bufs=2, space="PSUM") as psp:

            # ---- load constants / weights (two packed DMAs) ----
            cpk = cst.tile([128, NCF], F32, name="cpk")
            nc.sync.dma_start(out=cpk, in_=cpack_d)
            bpk = cst.tile([128, NBF], BF16, name="bpk")
            nc.sync.dma_start(out=bpk, in_=bpack_d)

            class _Cur:
                def __init__(self):
                    self.o = 0
            _cf, _cb = _Cur(), _Cur()

            def fsl(n):
                s = cpk[:, _cf.o:_cf.o + n]
                _cf.o += n
                return s

            def bsl(n):
                s = bpk[:, _cb.o:_cb.o + n]
                _cb.o += n
                return s

            gm1b_t, gm1s_t, rep1_t, gm2_t, rep2_t = [], [], [], [], []
            for c in range(2):
                gm1b_t.append(fsl(32)); gm1s_t.append(fsl(32))
                rep1_t.append(fsl(128))
                gm2_t.append(fsl(16)); rep2_t.append(fsl(128))
            g3_t, r3_t = [], []
            for g in range(4):
                g3_t.append(fsl(2)); r3_t.append(fsl(128))
            g4_t = fsl(8); r4_t = fsl(64)
            b0_t = [fsl(1) for _ in range(2)]
            b1_t = [fsl(1) for _ in range(4)]
            b2_t = fsl(1)
            eps_t = fsl(1)
            w2_t = [fsl(32) for _ in range(2)]
            w3_t = [fsl(128) for _ in range(4)]
            assert _cf.o == NCF
            w0_t = [bsl(256) for _ in range(2)]
            w1_t = [[bsl(128) for _ in range(6)] for _ in range(4)]
            wd_t = [bsl(256) for _ in range(4)]
            assert _cb.o == NBF

            # shared junk output for tensor_tensor_reduce
            junk = cst.tile([128, 2048], F32, name="ttr_junk")

            # ------------------------------------------------ helpers
            def silu_evac(out_ap, psum_ap, bias_ap, tag, accum=None):
                """out = silu(psum + bias) [+ per-partition row-sum]."""
                acc = accum if USE_ACT_ACCUM else None
                if not sim_safe:
                    nc.scalar.activation(out=out_ap, in_=psum_ap, func=AF.Silu,
                                         bias=bias_ap, scale=1.0,
                                         accum_out=acc)
                    if accum is not None and not USE_ACT_ACCUM:
                        nc.vector.tensor_reduce(out=accum, in_=out_ap,
                                                axis=AX.X, op=ALU.add)
                else:
                    sgf = sm.tile([128, 1024], F32, tag="sg", bufs=1,
                                  name=f"sg_{tag}", uniquify=True)
                    pp = psum_ap.partition_size()
                    ff = psum_ap.free_size()
                    sgt = sgf[0:pp, 0:ff]
                    nc.scalar.activation(out=sgt, in_=psum_ap, func=AF.Sigmoid,
                                         bias=bias_ap, scale=1.0)
                    nc.vector.scalar_tensor_tensor(
                        out=out_ap, in0=psum_ap, scalar=bias_ap, in1=sgt,
                        op0=ALU.add, op1=ALU.mult, accum_out=accum)
                    acc = accum  # sim path: STT accum is fine

            def gn_scale_bias(mvs, gmat_list, rmat_list, ngroups, tag,
                              ncols=2, sums=False):
                """Per-channel (scale, bias[, -mean]) tiles for a group norm.

                mvs: [128,2] tiles per chunk; bn-mode: (mean, var);
                sums-mode: (S1, S2) with 1/(gs*N) baked into gmat.
                """
                nchunk = len(mvs)
                rstats = []
                for ci, mv in enumerate(mvs):
                    if sums:
                        rstats.append(mv)
                        continue
                    r = sm.tile([128, 2], F32, tag=f"r_{tag}", bufs=2 * nchunk)
                    nc.vector.tensor_copy(out=r[:, 0:1], in_=mv[:, 0:1])
                    nc.vector.scalar_tensor_tensor(
                        out=r[:, 1:2], in0=mv[:, 0:1], scalar=mv[:, 0:1],
                        in1=mv[:, 1:2], op0=ALU.mult, op1=ALU.add)
                    rstats.append(r)
                pg = psp.tile([128, 2], F32, tag="gn_ps", bufs=1)
                for ci in range(nchunk):
                    nc.tensor.matmul(pg[0:ngroups, :], gmat_list[ci],
                                     rstats[ci],
                                     start=(ci == 0), stop=(ci == nchunk - 1))
                gt = sm.tile([128, 2], F32, tag=f"gt_{tag}", bufs=2)
                nc.vector.memset(gt, 0.0)
                nc.scalar.copy(out=gt[0:ngroups, :], in_=pg[0:ngroups, :])
                # -var = mean^2 - E[x^2]
                negv = sm.tile([128, 1], F32, tag=f"nv_{tag}", bufs=2)
                nc.vector.scalar_tensor_tensor(
                    out=negv[0:ngroups], in0=gt[0:ngroups, 0:1],
                    scalar=gt[0:ngroups, 0:1], in1=gt[0:ngroups, 1:2],
                    op0=ALU.mult, op1=ALU.subtract)
                sd = sm.tile([128, 1], F32, tag=f"sd_{tag}", bufs=2)
                nc.scalar.activation(out=sd[0:ngroups], in_=negv[0:ngroups],
                                     func=AF.Sqrt, bias=eps_t[0:ngroups],
                                     scale=-1.0)
                rstd = sm.tile([128, 1], F32, tag=f"rs_{tag}", bufs=2)
                nc.vector.reciprocal(out=rstd[0:ngroups], in_=sd[0:ngroups])
                stg = sm.tile([128, 3], F32, tag=f"st_{tag}", bufs=2)
                nc.vector.memset(stg, 0.0)
                nc.vector.tensor_copy(out=stg[0:ngroups, 0:1],
                                      in_=rstd[0:ngroups])
                nc.vector.tensor_scalar(
                    out=stg[0:ngroups, 1:2], in0=gt[0:ngroups, 0:1],
                    scalar1=rstd[0:ngroups], scalar2=-1.0,
                    op0=ALU.mult, op1=ALU.mult)
                if ncols == 3:
                    nc.vector.tensor_scalar(
                        out=stg[0:ngroups, 2:3], in0=gt[0:ngroups, 0:1],
                        scalar1=-1.0, scalar2=None, op0=ALU.mult)
                scs = []
                for ci, rmat in enumerate(rmat_list):
                    mm = rmat.shape[-1]
                    pr = psp.tile([128, 3], F32, tag="gn_ps", bufs=1)
                    nc.tensor.matmul(pr[0:mm, 0:ncols], rmat,
                                     stg[:, 0:ncols], start=True, stop=True)
                    sc = sm.tile([128, 3], F32, tag=f"sc_{tag}",
                                 bufs=2 * nchunk)
                    nc.scalar.copy(out=sc[0:mm, 0:ncols], in_=pr[0:mm, 0:ncols])
                    scs.append(sc)
                return scs

            def sumsq_vector(src, s2_out, tag):
                """S2 = sum(src^2) over 4096 free elems, via 2 chained TTRs."""
                if USE_TTR:
                    h1 = sm.tile([128, 1], F32, tag="sqv", bufs=4,
                                 uniquify=True, name=f"sqv_{tag}")
                    nc.vector.tensor_tensor_reduce(
                        out=junk, in0=src[:, 0:2048], in1=src[:, 0:2048],
                        scale=1.0, scalar=0.0, op0=ALU.mult, op1=ALU.add,
                        accum_out=h1)
                    nc.vector.tensor_tensor_reduce(
                        out=junk, in0=src[:, 2048:4096], in1=src[:, 2048:4096],
                        scale=1.0, scalar=h1, op0=ALU.mult, op1=ALU.add,
                        accum_out=s2_out)
                    return
                parts = sm.tile([128, 2], F32, tag="sqv2", bufs=4,
                                uniquify=True, name=f"sqv2_{tag}")
                jb2 = junk.bitcast(BF16)
                for h in range(2):
                    nc.vector.tensor_tensor(
                        out=jb2[:, 0:2048], in0=src[:, h * 2048:(h + 1) * 2048],
                        in1=src[:, h * 2048:(h + 1) * 2048], op=ALU.mult)
                    nc.vector.tensor_reduce(out=parts[:, h:h + 1],
                                            in_=jb2[:, 0:2048],
                                            axis=AX.X, op=ALU.add)
                nc.vector.tensor_reduce(out=s2_out, in_=parts,
                                        axis=AX.X, op=ALU.add)

            def sumsq_scalar(src, s2_out, tag):
                """S2 via 2 scalar Square activations with accumulators."""
                if not USE_ACT_ACCUM:
                    sumsq_vector(src, s2_out, tag)
                    return
                parts = sm.tile([128, 2], F32, tag="sqp", bufs=4,
                                uniquify=True, name=f"sqs_{tag}")
                jb = junk.bitcast(BF16)
                for h in range(2):
                    nc.scalar.activation(
                        out=jb[:, 0:2048], in_=src[:, h * 2048:(h + 1) * 2048],
                        func=AF.Square, scale=1.0,
                        accum_out=parts[:, h:h + 1])
                nc.vector.tensor_reduce(out=s2_out, in_=parts,
                                        axis=AX.X, op=ALU.add)

            # ------------------------------------------------ per-sample state
            st = [dict() for _ in range(BPC)]

            def phase_load(b):
                s = st[b]
                hsv = hsb[b].rearrange("c h w -> c (h w)")
                s["xw"] = [big.tile([128, NPIX], BF16, tag="xw", bufs=2,
                                    name=f"xw{b}_{i}") for i in range(2)]
                for c in range(2):
                    for qd in range(4):
                        ql = bass.ts(qd, 1024)
                        nc.sync.dma_start(
                            out=s["xw"][c][:, ql],
                            in_=hsv[c * 128:(c + 1) * 128, ql])

            def phase_gn1(b):
                s = st[b]
                bst1 = [sm.tile([128, NT, 6], F32, tag="bst1", bufs=4,
                                name=f"bst1_{b}_{i}") for i in range(2)]
                for c in range(2):
                    for n in range(NT):
                        nc.vector.bn_stats(out=bst1[c][:, n, :],
                                           in_=s["xw"][c][:, bass.ts(n, 512)])
                mv1 = []
                for c in range(2):
                    mv = sm.tile([128, 2], F32, tag="mv1", bufs=4,
                                 name=f"mv1_{b}_{c}")
                    nc.vector.bn_aggr(out=mv, in_=bst1[c])
                    mv1.append(mv)
                s["mv1"] = mv1

            def phase_w0fold(b):
                s = st[b]
                sc1 = gn_scale_bias(s["mv1"], gm1b_t, rep1_t, 32, "gn1")
                w0s = [sm.tile([128, 256], BF16, tag="w0s", bufs=4,
                               name=f"w0s{b}_{i}") for i in range(2)]
                t1b = [sm.tile([128, 1], BF16, tag="t1b", bufs=4,
                               name=f"t1b{b}_{i}") for i in range(2)]
                for c in range(2):
                    nc.vector.tensor_scalar_mul(out=w0s[c], in0=w0_t[c],
                                                scalar1=sc1[c][:, 0:1])
                    nc.vector.tensor_copy(out=t1b[c], in_=sc1[c][:, 1:2])
                b0p = [sm.tile([128, 1], F32, tag="b0p", bufs=4,
                               name=f"b0p{b}_{i}") for i in range(2)]
                for m in range(2):
                    pb = psp.tile([128, 1], F32, tag="gn_ps", bufs=1)
                    for kc in range(2):
                        nc.tensor.matmul(
                            pb, w0s[kc][:, m * 128:(m + 1) * 128], t1b[kc],
                            start=(kc == 0), stop=(kc == 1))
                    nc.scalar.activation(out=b0p[m], in_=pb,
                                         func=AF.Identity, bias=b0_t[m],
                                         scale=1.0)
                s["w0s"], s["b0p"] = w0s, b0p

            def phase_conv0(b):
                s = st[b]
                y0 = [big.tile([128, NPIX], BF16, tag="y0", bufs=2,
                               name=f"y0{b}_{i}") for i in range(2)]
                acc0 = [sm.tile([128, 4], F32, tag="acc0", bufs=4,
                                name=f"acc0_{b}_{i}") for i in range(2)]
                for m in range(2):
                    for q in range(4):   # 1024-pix strips
                        pt = psp.tile([128, 1024], F32, tag="acc", bufs=3,
                                      name=f"pc0_{b}_{m}_{q}", uniquify=True)
                        for hh in range(2):
                            n = q * 2 + hh
                            for kc in range(2):
                                nc.tensor.matmul(
                                    pt[:, hh * 512:(hh + 1) * 512],
                                    s["w0s"][kc][:, m * 128:(m + 1) * 128],
                                    s["xw"][kc][:, bass.ts(n, 512)],
                                    start=(kc == 0), stop=(kc == 1))
                        silu_evac(y0[m][:, q * 1024:(q + 1) * 1024], pt,
                                  s["b0p"][m], f"c0_{b}",
                                  accum=acc0[m][:, q:q + 1])
                s["y0"], s["acc0"] = y0, acc0

            def phase_gn2(b):
                s = st[b]
                mv2 = []
                for c in range(2):
                    mv = sm.tile([128, 2], F32, tag="mv2", bufs=4,
                                 name=f"mv2_{b}_{c}")
                    nc.vector.tensor_reduce(out=mv[:, 0:1], in_=s["acc0"][c],
                                            axis=AX.X, op=ALU.add)
                    sumsq_scalar(s["y0"][c], mv[:, 1:2], f"g2_{b}_{c}")
                    mv2.append(mv)
                s["sc2"] = gn_scale_bias(mv2, gm2_t, rep2_t, 16, "gn2",
                                         sums=True)

            def phase_xp(b):
                """Build per-group padded input tiles A (x | x-shift-dx+1)
                and B (x | x-shift-dy+1)."""
                s = st[b]
                At = [big.tile([128, PADW, PADW], BF16, tag="Atile", bufs=4,
                               name=f"A{b}_{g}") for g in range(4)]
                for g in range(4):
                    a = At[g]
                    # group g's unshifted x lives at partition base xb so the
                    # gn2-apply (vector: no cross-partition path) stays
                    # base-aligned with y0; shifted copies move via DMA.
                    xb = (g % 2) * 64
                    sb = 64 - xb
                    ax = a[xb:xb + 64]
                    # zero borders of the x half (rows 0,65; cols 0,65)
                    nc.gpsimd.memset(ax[:, 0:1, :], 0.0)
                    nc.gpsimd.memset(ax[:, PADW - 1:PADW, :], 0.0)
                    nc.gpsimd.memset(ax[:, 1:PADW - 1, 0:1], 0.0)
                    nc.gpsimd.memset(ax[:, 1:PADW - 1, PADW - 1:PADW], 0.0)
                    # gn2 apply: y0 rows of this group's in-channels
                    src = s["y0"][g // 2]
                    sc2 = s["sc2"][g // 2]
                    nc.vector.tensor_scalar(
                        out=ax[:, 1:H + 1, 1:W + 1],
                        in0=src[xb:xb + 64, :].rearrange(
                            "p (h w) -> p h w", h=H),
                        scalar1=sc2[xb:xb + 64, 0:1],
                        scalar2=sc2[xb:xb + 64, 1:2],
                        op0=ALU.mult, op1=ALU.add)
                    # A shifted half: x shifted one col left (DMA cross-move)
                    nc.gpsimd.memset(a[sb:sb + 64, :, PADW - 1:PADW], 0.0)
                    nc.sync.dma_start(out=a[sb:sb + 64, :, 0:PADW - 1],
                                      in_=ax[:, :, 1:PADW])
                s["A"] = At

            def phase_conv1(b):
                s = st[b]
                y1 = [big.tile([128, NPIX], BF16, tag="y1", bufs=8,
                               name=f"y1{b}_{g}") for g in range(4)]
                acc1 = [sm.tile([128, 4], F32, tag="acc1", bufs=4,
                                name=f"acc1_{b}_{g}") for g in range(4)]
                pooled = sm.tile([128, 4, Hn * Wn], BF16, tag="pooled",
                                 bufs=2, name=f"pool{b}")
                mv3 = [sm.tile([128, 2], F32, tag="mv3", bufs=4,
                               name=f"mv3_{b}_{g}") for g in range(4)]
                for g in range(4):
                    a = s["A"][g]
                    for q in range(4):     # 1024-pix strips (16 rows)
                        pt = psp.tile([128, 1024], F32, tag="acc", bufs=3,
                                      name=f"pc1_{b}_{g}_{q}", uniquify=True)
                        for hh in range(2):
                            r0 = (q * 2 + hh) * WS
                            po = pt[:, hh * 512:(hh + 1) * 512]
                            for i, dy in enumerate((-1, 0, 1)):
                                nc.tensor.matmul(
                                    po, w1_t[g][3 + i],
                                    a[:, r0 + dy + 1:r0 + dy + 9, 2:W + 2],
                                    start=(i == 0), stop=False)
                            for i, dy in enumerate((-1, 0, 1)):
                                nc.tensor.matmul(
                                    po, w1_t[g][i],
                                    a[:, r0 + dy + 1:r0 + dy + 9, 0:W],
                                    start=False, stop=(i == 2))
                        silu_evac(y1[g][:, q * 1024:(q + 1) * 1024], pt,
                                  b1_t[g], f"c1_{b}",
                                  accum=acc1[g][:, q:q + 1])
                    # window pooling: tree adds (2-byte 2x mode), partials
                    # live in bf16 regions of the shared junk scratch
                    yv = y1[g].rearrange("p (a ws) -> p a ws", ws=WS)
                    jb = junk.bitcast(BF16)
                    t1 = jb[:, 0:2048].rearrange("p (a b) -> p a b", b=4)
                    nc.vector.tensor_tensor(out=t1, in0=yv[:, :, 0:4],
                                            in1=yv[:, :, 4:8], op=ALU.add)
                    t2 = jb[:, 2048:3072].rearrange("p (a b) -> p a b", b=2)
                    nc.vector.tensor_tensor(out=t2, in0=t1[:, :, 0:2],
                                            in1=t1[:, :, 2:4], op=ALU.add)
                    pa = jb[:, 3072:3584]
                    nc.vector.tensor_tensor(out=pa.rearrange("p (h wn) -> p h wn", wn=Wn),
                                            in0=t2[:, :, 0:1].squeeze(2).rearrange("p (h wn) -> p h wn", wn=Wn),
                                            in1=t2[:, :, 1:2].squeeze(2).rearrange("p (h wn) -> p h wn", wn=Wn),
                                            op=ALU.add)
                    # h-direction tree over h2: keep 3D by slicing the
                    # contiguous (h2 wn) tail of each hn row
                    pv = pa.rearrange("p (hn x) -> p hn x", hn=Hn)  # x=h2*wn
                    u1 = jb[:, 3584:3840].rearrange("p (hn x) -> p hn x",
                                                    hn=Hn)          # x=32
                    nc.vector.tensor_tensor(out=u1, in0=pv[:, :, 0:32],
                                            in1=pv[:, :, 32:64], op=ALU.add)
                    u2 = jb[:, 3840:3968].rearrange("p (hn x) -> p hn x",
                                                    hn=Hn)          # x=16
                    nc.vector.tensor_tensor(out=u2, in0=u1[:, :, 0:16],
                                            in1=u1[:, :, 16:32], op=ALU.add)
                    nc.vector.tensor_tensor(
                        out=pooled[:, g, :].rearrange("p (a b) -> p a b",
                                                      b=Wn),
                        in0=u2[:, :, 0:8], in1=u2[:, :, 8:16], op=ALU.add)
                    # GN3 stats
                    nc.vector.tensor_reduce(out=mv3[g][:, 0:1], in_=acc1[g],
                                            axis=AX.X, op=ALU.add)
                    sumsq_scalar(y1[g], mv3[g][:, 1:2], f"g3_{b}_{g}")
                s["y1"], s["pooled"], s["mv3"] = y1, pooled, mv3

            def phase_gn3(b):
                s = st[b]
                s["sc3"] = gn_scale_bias(s["mv3"], g3_t, r3_t, 2, "gn3",
                                         ncols=3, sums=True)

            def phase_attn(b):
                s = st[b]
                sc3 = s["sc3"]
                pooled = s["pooled"]
                # Radix amax: partners sit in opposite partition halves, so
                # stage the other half across with DMA (engines cannot move
                # data across partitions), then max at matching base.
                # am[0] rows = c 0:128 (groups 0,1), am[1] = c 128:256.
                am = [sm.tile([128, 64], F32, tag="am", bufs=2,
                              name=f"am{b}_{i}") for i in range(2)]
                stage = sm.tile([128, 2, 64], BF16, tag="pstage", bufs=2,
                                name=f"pstage_{b}")
                # r1 halves of even chunks -> base 0 (cols 0,1 = g 0,2)
                pv4 = pooled.rearrange("p (i j) w -> p i j w", i=2)
                nc.sync.dma_start(out=stage[0:64],
                                  in_=pv4[64:128, :, 0, :])
                # r0 halves of odd chunks -> base 64 (cols 0,1 = g 1,3)
                nc.sync.dma_start(out=stage[64:128],
                                  in_=pv4[0:64, :, 1, :])
                for g in range(4):
                    xb = (g % 2) * 64
                    pg = pooled[xb:xb + 64, g, :]
                    dst = am[g // 2][xb:xb + 64, :]
                    nc.vector.tensor_tensor(out=dst, in0=pg,
                                            in1=stage[xb:xb + 64, g // 2, :],
                                            op=ALU.max)
                    # normalize: am*(rstd3/64) + t3  (scalars uniform in group)
                    s64 = sm.tile([128, 1], F32, tag="s64", bufs=2,
                                  name=f"s64_{b}_{g}", uniquify=True)
                    nc.vector.tensor_scalar(
                        out=s64[xb:xb + 64], in0=sc3[g][xb:xb + 64, 0:1],
                        scalar1=1.0 / (WS * WS), scalar2=None, op0=ALU.mult)
                    nc.vector.tensor_scalar(
                        out=dst, in0=dst, scalar1=s64[xb:xb + 64],
                        scalar2=sc3[g][xb:xb + 64, 1:2],
                        op0=ALU.mult, op1=ALU.add)
                # conv2 (1x1 g=2, 256->64) + silu
                p2 = psp.tile([128, 64], F32, tag="tp", bufs=1)
                for g in range(2):
                    nc.tensor.matmul(p2[g * 32:(g + 1) * 32, :], w2_t[g],
                                     am[g], start=True, stop=True)
                a2 = sm.tile([128, 64], F32, tag="a2", bufs=2)
                nc.vector.memset(a2, 0.0)
                silu_evac(a2[0:64, :], p2[0:64, :], b2_t[0:64], f"c2_{b}")
                # GN4
                mv4pad = sm.tile([128, 2], F32, tag="mv4", bufs=2)
                nc.vector.memset(mv4pad, 0.0)
                bst4 = sm.tile([128, 1, 6], F32, tag="bst4", bufs=2)
                nc.vector.bn_stats(out=bst4[0:64], in_=a2[0:64].unsqueeze(1))
                nc.vector.bn_aggr(out=mv4pad[0:64], in_=bst4[0:64])
                sc4 = gn_scale_bias([mv4pad], [g4_t], [r4_t], 8, "gn4")[0]
                a2n = sm.tile([128, 64], F32, tag="a2n", bufs=2)
                nc.vector.memset(a2n, 0.0)
                nc.vector.tensor_scalar(
                    out=a2n[0:64], in0=a2[0:64],
                    scalar1=sc4[0:64, 0:1], scalar2=sc4[0:64, 1:2],
                    op0=ALU.mult, op1=ALU.add)
                # conv3 (1x1 g=2, 64->512), b3 = 0; split each chunk's
                # matmul into its two radix halves (M=64 each) so the
                # subtract happens at matching partition base 0.
                dall = sm.tile([64, 4, 64], F32, tag="dall", bufs=1)
                for g in range(4):
                    p3 = psp.tile([64, 128], F32, tag="tp", bufs=1,
                                  name=f"p3_{b}_{g}", uniquify=True)
                    nc.tensor.matmul(p3[:, 0:64], w3_t[g][:, 0:64], a2n,
                                     start=True, stop=True)
                    nc.tensor.matmul(p3[:, 64:128], w3_t[g][:, 64:128], a2n,
                                     start=True, stop=True)
                    a3b = sm.tile([64, 64], F32, tag="a3b", bufs=2,
                                  name=f"a3b_{b}_{g}", uniquify=True)
                    nc.scalar.copy(out=a3b, in_=p3[:, 64:128])
                    nc.vector.tensor_tensor(out=dall[:, g, :],
                                            in0=p3[:, 0:64],
                                            in1=a3b,
                                            op=ALU.subtract)
                # sigma(+d) at base 0; DMA d up to base 64 for sigma(-d)
                sintP = sm.tile([64, 4, 64], BF16, tag="sintP", bufs=2,
                                name=f"sintP{b}")
                nc.scalar.activation(out=sintP, in_=dall,
                                     func=AF.Sigmoid, scale=1.0)
                dhi = sm.tile([128, 4, 64], F32, tag="dhi", bufs=2,
                              name=f"dhi{b}")
                nc.sync.dma_start(out=dhi[64:128], in_=dall)
                sintN = sm.tile([128, 4, 64], BF16, tag="sintN", bufs=2,
                                name=f"sintN{b}")
                nc.scalar.activation(out=sintN[64:128], in_=dhi[64:128],
                                     func=AF.Sigmoid, scale=-1.0)
                # expand gate over ws on gpsimd (3D, per partition half)
                gx = [sm.tile([128, Hn * Wn, WS], BF16, tag="gx", bufs=4,
                              name=f"gx{b}_{g}") for g in range(4)]
                for g in range(4):
                    nc.vector.tensor_copy(
                        out=gx[g][0:64],
                        in_=sintP[:, g, :].unsqueeze(2).broadcast_to(
                            [64, Hn * Wn, WS]))
                    nc.vector.tensor_copy(
                        out=gx[g][64:128],
                        in_=sintN[64:128, g, :].unsqueeze(2).broadcast_to(
                            [64, Hn * Wn, WS]))
                s["gx"] = gx

            def phase_gate(b):
                s = st[b]
                # fold GN3 rstd into final matmul weights
                wds = [sm.tile([128, 256], BF16, tag="wds", bufs=4,
                               name=f"wds{b}_{g}") for g in range(4)]
                for g in range(4):
                    nc.vector.tensor_scalar_mul(
                        out=wds[g], in0=wd_t[g], scalar1=s["sc3"][g][:, 0:1])
                # z = (y1 + (-mean3)) * gate, in place. Iterate h2 (not hn):
                # then the gate operand is the plain packed gx tile viewed
                # [128, hn, (wn ws)] with NO broadcast, so the STT keeps its
                # 2-byte fast mode (broadcast operands force 1 elem/cycle).
                for g in range(4):
                    yv = s["y1"][g].rearrange(
                        "p (hn h2 x) -> p hn h2 x", hn=Hn, h2=WS)
                    gv = s["gx"][g].rearrange(
                        "p (hn wn) ws -> p hn (wn ws)", hn=Hn)
                    for h2 in range(WS):
                        nc.vector.scalar_tensor_tensor(
                            out=yv[:, :, h2, :], in0=yv[:, :, h2, :],
                            scalar=s["sc3"][g][:, 2:3],
                            in1=gv, op0=ALU.add, op1=ALU.mult)
                s["wds"] = wds

            def phase_final(b):
                s = st[b]
                ot = [big.tile([128, NPIX], BF16, tag="ot", bufs=2,
                               name=f"ot{b}_{i}") for i in range(2)]
                acc5 = [sm.tile([128, 4], F32, tag="acc5", bufs=4,
                                name=f"acc5_{b}_{i}") for i in range(2)]
                mv5 = [sm.tile([128, 2], F32, tag="mv5", bufs=4,
                               name=f"mv5f_{b}_{i}") for i in range(2)]
                for m in range(2):
                    for q in range(4):
                        pt = psp.tile([128, 1024], F32, tag="acc", bufs=3,
                                      name=f"pcf_{b}_{m}_{q}", uniquify=True)
                        for hh in range(2):
                            n = q * 2 + hh
                            for kc in range(4):
                                nc.tensor.matmul(
                                    pt[:, hh * 512:(hh + 1) * 512],
                                    s["wds"][kc][:, m * 128:(m + 1) * 128],
                                    s["y1"][kc][:, bass.ts(n, 512)],
                                    start=(kc == 0), stop=(kc == 3))
                        if USE_ACT_ACCUM:
                            nc.scalar.activation(
                                out=ot[m][:, q * 1024:(q + 1) * 1024], in_=pt,
                                func=AF.Copy, accum_out=acc5[m][:, q:q + 1])
                        else:
                            nc.scalar.copy(
                                out=ot[m][:, q * 1024:(q + 1) * 1024], in_=pt)
                            nc.vector.tensor_reduce(
                                out=acc5[m][:, q:q + 1],
                                in_=ot[m][:, q * 1024:(q + 1) * 1024],
                                axis=AX.X, op=ALU.add)
                    # chunk stats immediately: the squares of chunk m
                    # overlap the next chunk's matmuls
                    nc.vector.tensor_reduce(out=mv5[m][:, 0:1],
                                            in_=acc5[m],
                                            axis=AX.X, op=ALU.add)
                    sumsq_scalar(ot[m], mv5[m][:, 1:2], f"g5_{b}_{m}")
                s["ot"], s["acc5"], s["mv5"] = ot, acc5, mv5

            def phase_gn5(b):
                s = st[b]
                s["sc5"] = gn_scale_bias(s["mv5"], gm1s_t, rep1_t, 32, "gn5",
                                         sums=True)

            def phase_out(b):
                s = st[b]
                ov = out_d[b].rearrange("c h w -> c (h w)")
                hsv = hsb[b].rearrange("c h w -> c (h w)")
                for c in range(2):
                    sc5 = s["sc5"][c]
                    for h2 in range(2):
                        sl = bass.ts(h2, 2048)
                        xr = sm.tile([128, 2048], BF16, tag="xr", bufs=2,
                                     name=f"xr{b}_{c}_{h2}", uniquify=True)
                        nc.sync.dma_start(
                            out=xr, in_=hsv[c * 128:(c + 1) * 128, sl])
                        ts = s["ot"][c][:, sl]
                        nc.vector.tensor_scalar(
                            out=ts, in0=ts, scalar1=sc5[:, 0:1],
                            scalar2=sc5[:, 1:2], op0=ALU.mult, op1=ALU.add)
                        nc.vector.tensor_tensor(
                            out=ts, in0=ts, in1=xr, op=ALU.add)
                        nc.sync.dma_start(
                            out=ov[c * 128:(c + 1) * 128, sl], in_=ts)

            def scoped(name, fn, *a):
                _s, _ = nc.enter_named_scope(name, False)
                fn(*a)
                nc.leave_named_scope(name, _s, False)

            # ---------------- interleaved schedule ----------------
            scoped("ld0", phase_load, 0)
            scoped("gn1_0", phase_gn1, 0)
            scoped("w0f_0", phase_w0fold, 0)
            scoped("ld1", phase_load, 1)
            scoped("conv0_0", phase_conv0, 0)
            scoped("gn1_1", phase_gn1, 1)
            scoped("gn2_0", phase_gn2, 0)
            scoped("xp_0", phase_xp, 0)
            scoped("conv1_0", phase_conv1, 0)
            scoped("w0f_1", phase_w0fold, 1)
            scoped("conv0_1", phase_conv0, 1)
            scoped("gn3_0", phase_gn3, 0)
            scoped("attn_0", phase_attn, 0)
            scoped("gn2_1", phase_gn2, 1)
            scoped("xp_1", phase_xp, 1)
            scoped("conv1_1", phase_conv1, 1)
            scoped("gate_0", phase_gate, 0)
            scoped("gn3_1", phase_gn3, 1)
            scoped("attn_1", phase_attn, 1)
            scoped("final_0", phase_final, 0)
            scoped("gn5_0", phase_gn5, 0)
            scoped("gate_1", phase_gate, 1)
            scoped("out_0", phase_out, 0)
            scoped("final_1", phase_final, 1)
            scoped("gn5_1", phase_gn5, 1)
            scoped("out_1", phase_out, 1)

    nc.compile()
    return nc


# ---------------------------------------------------------------- entry

_CACHE = {}


def _get_nc(sim_safe=False):
    key = bool(sim_safe)
    if key not in _CACHE:
        _CACHE[key] = build_nc(sim_safe=key)
    return _CACHE[key]


def make_in_maps(inputs):
    hs_full = np.ascontiguousarray(inputs["hidden_state"], dtype=np.float32)
    wd = _host_weights(
        np.asarray(inputs["w0"], np.float32), np.asarray(inputs["b0"], np.float32),
        np.asarray(inputs["w1"], np.float32), np.asarray(inputs["b1"], np.float32),
        np.asarray(inputs["w2"], np.float32), np.asarray(inputs["b2"], np.float32),
        np.asarray(inputs["w3"], np.float32), np.asarray(inputs["b3"], np.float32),
        np.asarray(inputs["weight"], np.float32))
    cm = _host_consts()
    cpack, bpack = _pack_consts(wd, cm)
    assert cpack.shape[1] == NCF, (cpack.shape, NCF)
    assert bpack.shape[1] == NBF, (bpack.shape, NBF)
    shared = {"cpack": cpack, "bpack": bpack}
    in_maps = []
    for i in range(NCORES):
        m = dict(shared)
        m["hsb"] = np.ascontiguousarray(
            hs_full[i * BPC:(i + 1) * BPC]).astype(ml_dtypes.bfloat16)
        in_maps.append(m)
    return in_maps


def kernel(**inputs):
    from concourse import bass_utils
    nc = _get_nc(sim_safe=False)
    in_maps = make_in_maps(inputs)
    res = bass_utils.run_bass_kernel_spmd(nc, in_maps,
                                          core_ids=list(range(NCORES)))
    out = np.concatenate([res.results[i]["out"] for i in range(NCORES)],
                         axis=0)
    return out.astype(np.float32)
